# revision 3
# baseline (speedup 1.0000x reference)
"""Gated self-attention Trainium2 kernel (8-core SPMD, data-parallel over batch).

Math (per batch b, head h, DH=64):
  h = concat(word, entity)                       [L=640, D=1024]
  q/k/v = h @ W + bias                           (7 projections)
  S^T[k,q] = sum_dh K[k,dh] Q[q,dh]              (scores, transposed layout)
  gate: word-word / ent-ent blocks: -5.0; cross blocks: -10*sigmoid(w2e/e2w)
  P = softmax_k((S + gate)/8 + mask)
  ctx = P^T @ V                                  -> [L, D] split back to word/entity

Device layout choices:
  - scores kept transposed [k, q]: k on partitions so the softmax denominator
    rides the ctx matmul as an extra all-ones column of V (row 64 of ctx PSUM).
  - gate cross blocks: -1.25*sigmoid(x) + 0.625 == 0.625*tanh(-x/2); the -0.5 is
    folded into the k-side gate projections, tanh applied at PSUM eviction, and
    5*I @ t accumulated into the scores PSUM (exp later scales by 0.125).
  - diagonal blocks' -5/8 + attention_mask ride the exp() per-partition bias.
"""

import os
import numpy as np

B, LW, LE, D, H = 16, 512, 128, 1024, 16
DH = D // H            # 64
L = LW + LE            # 640
P = 128
N_CORES = 8
BPC = B // N_CORES     # 2 batches per core
LC = L // P            # 5
DC = D // P            # 8
WC = LW // P           # 4

# matmul dtype: "fp32" | "bf16" | "fp32r"
MM_DTYPE = os.environ.get("BASSK_DTYPE", "fp32")

_CACHE = {}


def _nsplits(n):
    out, off = [], 0
    while off < n:
        w = min(512, n - off)
        out.append((off, w))
        off += w
    return out


def _build_nc():
    import concourse.bacc as bacc
    import concourse.mybir as mybir
    from concourse.tile import TileContext

    FP = mybir.dt.float32
    # SD: storage dtype of matmul operands in SBUF.
    SD = mybir.dt.bfloat16 if MM_DTYPE == "bf16" else FP
    AF = mybir.ActivationFunctionType

    nc = bacc.Bacc(None, target_bir_lowering=False, debug=False, num_devices=N_CORES)

    word = nc.dram_tensor("word", [BPC, LW, D], SD, kind="ExternalInput")
    ent = nc.dram_tensor("ent", [BPC, LE, D], SD, kind="ExternalInput")
    mask = nc.dram_tensor("mask", [BPC, L], FP, kind="ExternalInput")
    wnames = ["qw", "kw", "vw", "w2e_qw", "w2e_kw", "e2w_qw", "e2w_kw"]
    bnames = ["qb", "kb", "vb", "w2e_qb", "w2e_kb", "e2w_qb", "e2w_kb"]
    wd = {n: nc.dram_tensor(n, [D, D], SD, kind="ExternalInput") for n in wnames}
    bd = {n: nc.dram_tensor(n, [D], FP, kind="ExternalInput") for n in bnames}
    out_w = nc.dram_tensor("out_w", [BPC, LW, D], FP, kind="ExternalOutput")
    out_e = nc.dram_tensor("out_e", [BPC, LE, D], FP, kind="ExternalOutput")

    if MM_DTYPE == "fp32r":
        f32r = mybir.dt.float32r

        def mm(out, lhsT, rhs, **kw):
            nc.tensor.matmul(out, lhsT.bitcast(f32r), rhs.bitcast(f32r), **kw)
    else:
        def mm(out, lhsT, rhs, **kw):
            nc.tensor.matmul(out, lhsT, rhs, **kw)

    with TileContext(nc) as tc:
        with (
            tc.tile_pool(name="consts", bufs=1) as consts,
            tc.tile_pool(name="acts", bufs=1) as acts,
            tc.tile_pool(name="hload", bufs=2) as hload,
            tc.tile_pool(name="wpool", bufs=2) as wpool,
            tc.tile_pool(name="expp", bufs=2) as expp,
            tc.tile_pool(name="gatep", bufs=2) as gatep,
            tc.tile_pool(name="ev", bufs=3) as ev,
            tc.tile_pool(name="ps_big", bufs=2, space="PSUM") as ps_big,
            tc.tile_pool(name="ps_ctx", bufs=1, space="PSUM") as ps_ctx,
            tc.tile_pool(name="ps_tp", bufs=2, space="PSUM") as ps_tp,
        ):
            # ---- constants ----
            ident = consts.tile([P, P], SD, tag="ident")
            nc.gpsimd.memset(ident, 0.0)
            nc.gpsimd.affine_select(
                out=ident, in_=ident, compare_op=mybir.AluOpType.not_equal,
                fill=1.0, base=0, pattern=[[-1, P]], channel_multiplier=1,
            )
            id5 = consts.tile([P, P], SD, tag="id5")
            nc.gpsimd.memset(id5, 0.0)
            nc.gpsimd.affine_select(
                out=id5, in_=id5, compare_op=mybir.AluOpType.not_equal,
                fill=5.0, base=0, pattern=[[-1, P]], channel_multiplier=1,
            )
            identf = consts.tile([P, P], FP, tag="identf")
            nc.gpsimd.memset(identf, 0.0)
            nc.gpsimd.affine_select(
                out=identf, in_=identf, compare_op=mybir.AluOpType.not_equal,
                fill=1.0, base=0, pattern=[[-1, P]], channel_multiplier=1,
            )
            ones1 = consts.tile([1, P], SD, tag="ones1")
            nc.vector.memset(ones1, 1.0)
            vb_raw = consts.tile([1, D], FP, tag="vb_raw")
            nc.sync.dma_start(vb_raw, bd["vb"][None, :])
            vb_row = consts.tile([1, D], SD, tag="vb_row")
            nc.scalar.activation(vb_row, vb_raw, AF.Copy)

            # per-d_out biases striped [P, DC]; k-side gate biases pre-scaled by -0.5
            bias_sb = {}
            for n in bnames:
                t = consts.tile([P, DC], FP, tag=f"b_{n}")
                nc.sync.dma_start(t, bd[n].rearrange("(j p) -> p j", p=P))
                if n in ("w2e_kb", "e2w_kb"):
                    nc.scalar.mul(t, t, -0.5)
                bias_sb[n] = t

            for b in range(BPC):
                # ---- exp bias: mask[k] - 0.625, k striped on partitions ----
                ebias = acts.tile([P, LC], FP, tag="ebias")
                nc.sync.dma_start(ebias, mask[b].rearrange("(c p) -> p c", p=P))
                nc.vector.tensor_scalar_add(ebias, ebias, -0.625)

                # ---- h^T [d, l] ----
                hT = acts.tile([P, DC, L], SD, tag="hT")
                if MM_DTYPE == "bf16":
                    nc.sync.dma_start_transpose(hT[:, :, 0:LW], word[b])
                    nc.sync.dma_start_transpose(hT[:, :, LW:L], ent[b])
                else:
                    for lc in range(LC):
                        hch = hload.tile([P, D], SD, tag="hch")
                        if lc < WC:
                            nc.sync.dma_start(hch, word[b, P * lc:P * (lc + 1), :])
                        else:
                            nc.sync.dma_start(hch, ent[b])
                        for dc in range(DC):
                            pst = ps_tp.tile([P, P], FP, tag="tpf")
                            nc.tensor.transpose(pst, hch[:, P * dc:P * (dc + 1)], ident)
                            nc.scalar.activation(
                                hT[:, dc, P * lc:P * (lc + 1)], pst, AF.Copy)

                # ---- transposed projections: out[d_out, l] = W^T @ h^T ----
                qT = acts.tile([P, DC, L], SD, tag="qT")
                kT = acts.tile([P, DC, L], SD, tag="kT")
                w2eqT = acts.tile([P, DC, LW], SD, tag="w2eqT")
                w2ekT = acts.tile([P, DC, LE], SD, tag="w2ekT")
                e2wqT = acts.tile([P, DC, LE], SD, tag="e2wqT")
                e2wkT = acts.tile([P, DC, LW], SD, tag="e2wkT")

                projs = [
                    ("qw", "qb", 0, L, qT, 1.0),
                    ("kw", "kb", 0, L, kT, 1.0),
                    ("w2e_qw", "w2e_qb", 0, LW, w2eqT, 1.0),
                    ("w2e_kw", "w2e_kb", LW, L, w2ekT, -0.5),
                    ("e2w_qw", "e2w_qb", LW, L, e2wqT, 1.0),
                    ("e2w_kw", "e2w_kb", 0, LW, e2wkT, -0.5),
                ]
                QW = 256  # d_out columns per streamed weight tile
                for (wn, bn, l0, l1, outT, scale) in projs:
                    llen = l1 - l0
                    for q4 in range(D // QW):
                        wt = wpool.tile([P, DC, QW], SD, tag="w")
                        nc.sync.dma_start(
                            wt,
                            wd[wn].rearrange("(c p) o -> p c o", p=P)[
                                :, :, QW * q4:QW * (q4 + 1)],
                        )
                        for jj in range(QW // P):
                            j = (QW * q4) // P + jj
                            ps = ps_big.tile([P, L], FP, tag="big")
                            for c in range(DC):
                                for (n0, nl) in _nsplits(llen):
                                    mm(ps[:, n0:n0 + nl],
                                       wt[:, c, P * jj:P * (jj + 1)],
                                       hT[:, c, l0 + n0:l0 + n0 + nl],
                                       start=(c == 0), stop=(c == DC - 1))
                            nc.scalar.activation(
                                outT[:, j, 0:llen], ps[:, 0:llen], AF.Identity,
                                bias=bias_sb[bn][:, j:j + 1], scale=scale)

                # ---- v natural [l, 65*h] with trailing ones column per head ----
                v_sb = acts.tile([P, LC, H * (DH + 1)], SD, tag="v")
                nc.vector.memset(
                    v_sb.rearrange("p l (h c) -> p l h c", c=DH + 1)[:, :, :, DH:], 1.0)
                for q4 in range(D // QW):
                    vt = wpool.tile([P, DC, QW], SD, tag="w")
                    nc.sync.dma_start(
                        vt,
                        wd["vw"].rearrange("(c p) o -> p c o", p=P)[
                            :, :, QW * q4:QW * (q4 + 1)],
                    )
                    hpq = QW // DH  # heads per weight tile (4)
                    for lc in range(LC):
                        ps = ps_big.tile([P, L], FP, tag="big")
                        for c in range(DC):
                            mm(ps[:, 0:QW], hT[:, c, P * lc:P * (lc + 1)],
                               vt[:, c, :], start=(c == 0), stop=False)
                        mm(ps[:, 0:QW], ones1,
                           vb_row[:, QW * q4:QW * (q4 + 1)],
                           start=False, stop=True)
                        nc.scalar.activation(
                            v_sb[:, lc].rearrange("p (h c) -> p h c", c=DH + 1)[
                                :, hpq * q4:hpq * (q4 + 1), 0:DH],
                            ps[:, 0:QW].rearrange("p (h c) -> p h c", c=DH),
                            AF.Copy)

                # ---- attention per head ----
                for h in range(H):
                    p0 = DH * (h % 2)
                    dc = h // 2

                    def ks(a, c0, c1):
                        return a[p0:p0 + DH, dc, c0:c1]

                    # cross-block gates: t = tanh(-0.5 * gate_logits)
                    psg = ps_big.tile([P, L], FP, tag="big")
                    mm(psg[:, 0:LW], ks(w2ekT, 0, LE), ks(w2eqT, 0, LW),
                       start=True, stop=True)
                    t_w2e = gatep.tile([P, LW], SD, tag="tw")
                    nc.scalar.activation(t_w2e, psg[:, 0:LW], AF.Tanh)

                    psg2 = ps_big.tile([P, L], FP, tag="big")
                    for c in range(WC):
                        mm(psg2[:, P * c:P * (c + 1)],
                           ks(e2wkT, P * c, P * (c + 1)),
                           ks(e2wqT, 0, LE), start=True, stop=True)
                    t_e2w = gatep.tile([P, LW], SD, tag="te")
                    nc.scalar.activation(t_e2w, psg2[:, 0:LW], AF.Tanh)

                    ctx_ps = ps_ctx.tile([P, L], FP, tag="ctx")
                    for c in range(LC):
                        pss = ps_big.tile([P, L], FP, tag="big")
                        if c < WC:
                            mm(pss[:, 0:LW], ks(kT, P * c, P * (c + 1)),
                               ks(qT, 0, LW), start=True, stop=True)
                            mm(pss[:, LW:L], ks(kT, P * c, P * (c + 1)),
                               ks(qT, LW, L), start=True, stop=False)
                            mm(pss[:, LW:L], id5, t_e2w[:, P * c:P * (c + 1)],
                               start=False, stop=True)
                        else:
                            mm(pss[:, 0:LW], ks(kT, P * c, P * (c + 1)),
                               ks(qT, 0, LW), start=True, stop=False)
                            mm(pss[:, 0:LW], id5, t_w2e, start=False, stop=True)
                            mm(pss[:, LW:L], ks(kT, P * c, P * (c + 1)),
                               ks(qT, LW, L), start=True, stop=True)
                        es = expp.tile([P, L], SD, tag="es")
                        nc.scalar.activation(es, pss, AF.Exp,
                                             scale=0.125, bias=ebias[:, c:c + 1])
                        for (n0, nl) in _nsplits(L):
                            mm(ctx_ps[0:DH + 1, n0:n0 + nl],
                               v_sb[:, c, (DH + 1) * h:(DH + 1) * (h + 1)],
                               es[:, n0:n0 + nl],
                               start=(c == 0), stop=(c == LC - 1))

                    ctxT = ev.tile([DH + 1, L], FP, tag="ctxT")
                    nc.scalar.activation(ctxT, ctx_ps[0:DH + 1, :], AF.Copy)
                    for qt in range(LC):
                        pst = ps_tp.tile([P, P], FP, tag="tpf")
                        nc.tensor.transpose(pst[:, 0:DH + 1],
                                            ctxT[:, P * qt:P * (qt + 1)],
                                            identf[0:DH + 1, 0:DH + 1])
                        inv = ev.tile([P, 1], FP, tag="inv")
                        nc.vector.reciprocal(inv, pst[:, DH:DH + 1])
                        ot = ev.tile([P, DH], FP, tag="ot")
                        nc.scalar.activation(ot, pst[:, 0:DH], AF.Identity, scale=inv)
                        if qt < WC:
                            nc.sync.dma_start(
                                out_w[b, P * qt:P * (qt + 1), DH * h:DH * (h + 1)], ot)
                        else:
                            nc.sync.dma_start(
                                out_e[b, :, DH * h:DH * (h + 1)], ot)

    nc.compile()
    return nc


def _get_nc():
    if "nc" not in _CACHE:
        _CACHE["nc"] = _build_nc()
    return _CACHE["nc"]


def kernel(word_hidden_states, entity_hidden_states, attention_mask, layer_num,
           qw, qb, kw, kb, vw, vb,
           w2e_qw, w2e_qb, w2e_kw, w2e_kb,
           e2w_qw, e2w_qb, e2w_kw, e2w_kb, **extra):
    nc = _get_nc()
    from concourse.bass_utils import run_bass_kernel_spmd

    if MM_DTYPE == "bf16":
        import ml_dtypes

        def cast(a):
            return np.asarray(a, np.float32).astype(ml_dtypes.bfloat16)
    else:
        def cast(a):
            return np.ascontiguousarray(np.asarray(a, np.float32))

    def f32(a):
        return np.ascontiguousarray(np.asarray(a, np.float32))

    word = cast(word_hidden_states)
    ent = cast(entity_hidden_states)
    mask = f32(attention_mask).reshape(B, L)
    weights = {
        "qw": cast(qw), "kw": cast(kw), "vw": cast(vw),
        "w2e_qw": cast(w2e_qw), "w2e_kw": cast(w2e_kw),
        "e2w_qw": cast(e2w_qw), "e2w_kw": cast(e2w_kw),
    }
    biases = {
        "qb": f32(qb), "kb": f32(kb), "vb": f32(vb),
        "w2e_qb": f32(w2e_qb), "w2e_kb": f32(w2e_kb),
        "e2w_qb": f32(e2w_qb), "e2w_kb": f32(e2w_kb),
    }

    in_maps = []
    for c in range(N_CORES):
        sl = slice(BPC * c, BPC * (c + 1))
        m = {"word": word[sl], "ent": ent[sl], "mask": mask[sl]}
        m.update(weights)
        m.update(biases)
        in_maps.append(m)

    res = run_bass_kernel_spmd(nc, in_maps, core_ids=list(range(N_CORES)))
    word_out = np.concatenate([res.results[c]["out_w"] for c in range(N_CORES)], axis=0)
    ent_out = np.concatenate([res.results[c]["out_e"] for c in range(N_CORES)], axis=0)
    return word_out, ent_out


# revision 5
# speedup vs baseline: 2.3933x; 2.3933x over previous
"""Gated self-attention Trainium2 kernel (8-core SPMD, data-parallel over batch).

Math (per batch b, head h, DH=64):
  h = concat(word, entity)                       [L=640, D=1024]
  q/k/v = h @ W + bias                           (7 projections)
  S^T[k,q] = sum_dh K[k,dh] Q[q,dh]              (scores, transposed layout)
  gate: word-word / ent-ent blocks: -5.0; cross blocks: -10*sigmoid(w2e/e2w)
  P = softmax_k((S + gate)/8 + mask)
  ctx = P^T @ V                                  -> [L, D] split back to word/entity

Device layout choices:
  - scores kept transposed [k, q]: k on partitions so the softmax denominator
    rides the ctx matmul as an extra all-ones column of V (row 64 of ctx PSUM).
  - gate cross blocks: -1.25*sigmoid(x) + 0.625 == 0.625*tanh(-x/2); the -0.5 is
    folded into the k-side gate projections, tanh applied at PSUM eviction, and
    5*I @ t accumulated into the scores PSUM (exp later scales by 0.125).
  - diagonal blocks' -5/8 + attention_mask ride the exp() per-partition bias.
"""

import os
import numpy as np

B, LW, LE, D, H = 16, 512, 128, 1024, 16
DH = D // H            # 64
L = LW + LE            # 640
P = 128
N_CORES = 8
BPC = B // N_CORES     # 2 batches per core
LC = L // P            # 5
DC = D // P            # 8
WC = LW // P           # 4

# matmul dtype: "fp32" | "bf16" | "fp32r"
MM_DTYPE = os.environ.get("BASSK_DTYPE", "fp32")

_CACHE = {}


def _nsplits(n):
    out, off = [], 0
    while off < n:
        w = min(512, n - off)
        out.append((off, w))
        off += w
    return out


def _build_nc():
    import concourse.bacc as bacc
    import concourse.mybir as mybir
    from concourse.tile import TileContext

    FP = mybir.dt.float32
    # SD: storage dtype of matmul operands in SBUF.
    SD = mybir.dt.bfloat16 if MM_DTYPE == "bf16" else FP
    AF = mybir.ActivationFunctionType

    nc = bacc.Bacc(None, target_bir_lowering=False, debug=False, num_devices=N_CORES)

    word = nc.dram_tensor("word", [BPC, LW, D], SD, kind="ExternalInput")
    ent = nc.dram_tensor("ent", [BPC, LE, D], SD, kind="ExternalInput")
    mask = nc.dram_tensor("mask", [BPC, L], FP, kind="ExternalInput")
    wnames = ["qw", "kw", "vw", "w2e_qw", "w2e_kw", "e2w_qw", "e2w_kw"]
    bnames = ["qb", "kb", "vb", "w2e_qb", "w2e_kb", "e2w_qb", "e2w_kb"]
    wd = {n: nc.dram_tensor(n, [D, D], SD, kind="ExternalInput") for n in wnames}
    bd = {n: nc.dram_tensor(n, [D], FP, kind="ExternalInput") for n in bnames}
    out_w = nc.dram_tensor("out_w", [BPC, LW, D], FP, kind="ExternalOutput")
    out_e = nc.dram_tensor("out_e", [BPC, LE, D], FP, kind="ExternalOutput")

    if MM_DTYPE == "fp32r":
        f32r = mybir.dt.float32r

        def mm(out, lhsT, rhs, **kw):
            nc.tensor.matmul(out, lhsT.bitcast(f32r), rhs.bitcast(f32r), **kw)
    else:
        def mm(out, lhsT, rhs, **kw):
            nc.tensor.matmul(out, lhsT, rhs, **kw)

    with TileContext(nc) as tc:
        with (
            tc.tile_pool(name="consts", bufs=1) as consts,
            tc.tile_pool(name="acts", bufs=1) as acts,
            tc.tile_pool(name="hload", bufs=2) as hload,
            tc.tile_pool(name="wpool", bufs=2) as wpool,
            tc.tile_pool(name="expp", bufs=2) as expp,
            tc.tile_pool(name="gatep", bufs=2) as gatep,
            tc.tile_pool(name="ev", bufs=3) as ev,
            tc.tile_pool(name="ps_big", bufs=2, space="PSUM") as ps_big,
            tc.tile_pool(name="ps_ctx", bufs=1, space="PSUM") as ps_ctx,
            tc.tile_pool(name="ps_tp", bufs=2, space="PSUM") as ps_tp,
        ):
            # ---- constants ----
            ident = consts.tile([P, P], SD, tag="ident")
            nc.gpsimd.memset(ident, 0.0)
            nc.gpsimd.affine_select(
                out=ident, in_=ident, compare_op=mybir.AluOpType.not_equal,
                fill=1.0, base=0, pattern=[[-1, P]], channel_multiplier=1,
            )
            identf = consts.tile([P, P], FP, tag="identf")
            nc.gpsimd.memset(identf, 0.0)
            nc.gpsimd.affine_select(
                out=identf, in_=identf, compare_op=mybir.AluOpType.not_equal,
                fill=1.0, base=0, pattern=[[-1, P]], channel_multiplier=1,
            )
            ones1 = consts.tile([1, P], SD, tag="ones1")
            nc.vector.memset(ones1, 1.0)
            vb_raw = consts.tile([1, D], FP, tag="vb_raw")
            nc.sync.dma_start(vb_raw, bd["vb"][None, :])
            vb_row = consts.tile([1, D], SD, tag="vb_row")
            nc.scalar.activation(vb_row, vb_raw, AF.Copy)

            # per-d_out biases striped [P, DC]; k-side gate biases pre-scaled by -0.5
            bias_sb = {}
            for n in bnames:
                t = consts.tile([P, DC], FP, tag=f"b_{n}")
                nc.sync.dma_start(t, bd[n].rearrange("(j p) -> p j", p=P))
                if n in ("w2e_kb", "e2w_kb"):
                    nc.scalar.mul(t, t, -0.5)
                bias_sb[n] = t

            for b in range(BPC):
                # ---- exp bias: mask[k] - 0.625, k striped on partitions ----
                ebias = acts.tile([P, LC], FP, tag="ebias")
                nc.sync.dma_start(ebias, mask[b].rearrange("(c p) -> p c", p=P))
                nc.vector.tensor_scalar_add(ebias, ebias, -0.625)

                # ---- h^T [d, l] ----
                hT = acts.tile([P, DC, L], SD, tag="hT")
                if MM_DTYPE == "bf16":
                    nc.sync.dma_start_transpose(hT[:, :, 0:LW], word[b])
                    nc.sync.dma_start_transpose(hT[:, :, LW:L], ent[b])
                else:
                    for lc in range(LC):
                        hch = hload.tile([P, D], SD, tag="hch")
                        if lc < WC:
                            nc.sync.dma_start(hch, word[b, P * lc:P * (lc + 1), :])
                        else:
                            nc.sync.dma_start(hch, ent[b])
                        for dc in range(DC):
                            pst = ps_tp.tile([P, P], FP, tag="tpf")
                            nc.tensor.transpose(pst, hch[:, P * dc:P * (dc + 1)], ident)
                            nc.scalar.activation(
                                hT[:, dc, P * lc:P * (lc + 1)], pst, AF.Copy)

                # ---- transposed projections: out[d_out, l] = W^T @ h^T ----
                qT = acts.tile([P, DC, L], SD, tag="qT")
                kT = acts.tile([P, DC, L], SD, tag="kT")
                w2eqT = acts.tile([P, DC, LW], SD, tag="w2eqT")
                w2ekT = acts.tile([P, DC, LE], SD, tag="w2ekT")
                e2wqT = acts.tile([P, DC, LE], SD, tag="e2wqT")
                e2wkT = acts.tile([P, DC, LW], SD, tag="e2wkT")

                projs = [
                    ("qw", "qb", 0, L, qT, 1.0),
                    ("kw", "kb", 0, L, kT, 1.0),
                    ("w2e_qw", "w2e_qb", 0, LW, w2eqT, 1.0),
                    ("w2e_kw", "w2e_kb", LW, L, w2ekT, -0.5),
                    ("e2w_qw", "e2w_qb", LW, L, e2wqT, 1.0),
                    ("e2w_kw", "e2w_kb", 0, LW, e2wkT, -0.5),
                ]
                QW = 256  # d_out columns per streamed weight tile
                for (wn, bn, l0, l1, outT, scale) in projs:
                    llen = l1 - l0
                    for q4 in range(D // QW):
                        wt = wpool.tile([P, DC, QW], SD, tag="w")
                        nc.sync.dma_start(
                            wt,
                            wd[wn].rearrange("(c p) o -> p c o", p=P)[
                                :, :, QW * q4:QW * (q4 + 1)],
                        )
                        for jj in range(QW // P):
                            j = (QW * q4) // P + jj
                            ps = ps_big.tile([P, L], FP, tag="big")
                            for c in range(DC):
                                for (n0, nl) in _nsplits(llen):
                                    mm(ps[:, n0:n0 + nl],
                                       wt[:, c, P * jj:P * (jj + 1)],
                                       hT[:, c, l0 + n0:l0 + n0 + nl],
                                       start=(c == 0), stop=(c == DC - 1))
                            nc.scalar.activation(
                                outT[:, j, 0:llen], ps[:, 0:llen], AF.Identity,
                                bias=bias_sb[bn][:, j:j + 1], scale=scale)

                # ---- v natural [l, 65*h] with trailing ones column per head ----
                v_sb = acts.tile([P, LC, H * (DH + 1)], SD, tag="v")
                nc.vector.memset(
                    v_sb.rearrange("p l (h c) -> p l h c", c=DH + 1)[:, :, :, DH:], 1.0)
                for q4 in range(D // QW):
                    vt = wpool.tile([P, DC, QW], SD, tag="w")
                    nc.sync.dma_start(
                        vt,
                        wd["vw"].rearrange("(c p) o -> p c o", p=P)[
                            :, :, QW * q4:QW * (q4 + 1)],
                    )
                    hpq = QW // DH  # heads per weight tile (4)
                    for lc in range(LC):
                        ps = ps_big.tile([P, L], FP, tag="big")
                        for c in range(DC):
                            mm(ps[:, 0:QW], hT[:, c, P * lc:P * (lc + 1)],
                               vt[:, c, :], start=(c == 0), stop=False)
                        mm(ps[:, 0:QW], ones1,
                           vb_row[:, QW * q4:QW * (q4 + 1)],
                           start=False, stop=True)
                        nc.scalar.activation(
                            v_sb[:, lc].rearrange("p (h c) -> p h c", c=DH + 1)[
                                :, hpq * q4:hpq * (q4 + 1), 0:DH],
                            ps[:, 0:QW].rearrange("p (h c) -> p h c", c=DH),
                            AF.Copy)

                # ---- attention per head ----
                for h in range(H):
                    p0 = DH * (h % 2)
                    dc = h // 2

                    def ks(a, c0, c1):
                        return a[p0:p0 + DH, dc, c0:c1]

                    # cross-block gates: t = tanh(-0.5 * gate_logits)
                    psg = ps_big.tile([P, L], FP, tag="big")
                    mm(psg[:, 0:LW], ks(w2ekT, 0, LE), ks(w2eqT, 0, LW),
                       start=True, stop=True)
                    t_w2e = gatep.tile([P, LW], SD, tag="tw")
                    nc.scalar.activation(t_w2e, psg[:, 0:LW], AF.Tanh)

                    psg2 = ps_big.tile([P, L], FP, tag="big")
                    for c in range(WC):
                        mm(psg2[:, P * c:P * (c + 1)],
                           ks(e2wkT, P * c, P * (c + 1)),
                           ks(e2wqT, 0, LE), start=True, stop=True)
                    t_e2w = gatep.tile([P, LW], SD, tag="te")
                    nc.scalar.activation(t_e2w, psg2[:, 0:LW], AF.Tanh)

                    ctx_ps = ps_ctx.tile([P, L], FP, tag="ctx")
                    for c in range(LC):
                        pss = ps_big.tile([P, L], FP, tag="big")
                        for (n0, nl) in _nsplits(L):
                            mm(pss[:, n0:n0 + nl], ks(kT, P * c, P * (c + 1)),
                               ks(qT, n0, n0 + nl), start=True, stop=True)
                        # gate cross block: pss += 5 * t  (on DVE, PE stays dense)
                        if c < WC:
                            nc.vector.scalar_tensor_tensor(
                                pss[:, LW:L], t_e2w[:, P * c:P * (c + 1)], 5.0,
                                pss[:, LW:L], mybir.AluOpType.mult,
                                mybir.AluOpType.add)
                        else:
                            nc.vector.scalar_tensor_tensor(
                                pss[:, 0:LW], t_w2e, 5.0,
                                pss[:, 0:LW], mybir.AluOpType.mult,
                                mybir.AluOpType.add)
                        es = expp.tile([P, L], SD, tag="es")
                        nc.scalar.activation(es, pss, AF.Exp,
                                             scale=0.125, bias=ebias[:, c:c + 1])
                        for (n0, nl) in _nsplits(L):
                            mm(ctx_ps[0:DH + 1, n0:n0 + nl],
                               v_sb[:, c, (DH + 1) * h:(DH + 1) * (h + 1)],
                               es[:, n0:n0 + nl],
                               start=(c == 0), stop=(c == LC - 1))

                    ctxT = ev.tile([DH + 1, L], FP, tag="ctxT")
                    nc.scalar.activation(ctxT, ctx_ps[0:DH + 1, :], AF.Copy)
                    for qt in range(LC):
                        pst = ps_tp.tile([P, P], FP, tag="tpf")
                        nc.tensor.transpose(pst[:, 0:DH + 1],
                                            ctxT[:, P * qt:P * (qt + 1)],
                                            identf[0:DH + 1, 0:DH + 1])
                        inv = ev.tile([P, 1], FP, tag="inv")
                        nc.vector.reciprocal(inv, pst[:, DH:DH + 1])
                        ot = ev.tile([P, DH], FP, tag="ot")
                        nc.scalar.activation(ot, pst[:, 0:DH], AF.Identity, scale=inv)
                        if qt < WC:
                            nc.sync.dma_start(
                                out_w[b, P * qt:P * (qt + 1), DH * h:DH * (h + 1)], ot)
                        else:
                            nc.sync.dma_start(
                                out_e[b, :, DH * h:DH * (h + 1)], ot)

    nc.compile()
    return nc


def _get_nc():
    if "nc" not in _CACHE:
        _CACHE["nc"] = _build_nc()
    return _CACHE["nc"]


def kernel(word_hidden_states, entity_hidden_states, attention_mask, layer_num,
           qw, qb, kw, kb, vw, vb,
           w2e_qw, w2e_qb, w2e_kw, w2e_kb,
           e2w_qw, e2w_qb, e2w_kw, e2w_kb, **extra):
    nc = _get_nc()
    from concourse.bass_utils import run_bass_kernel_spmd

    if MM_DTYPE == "bf16":
        import ml_dtypes

        def cast(a):
            return np.asarray(a, np.float32).astype(ml_dtypes.bfloat16)
    else:
        def cast(a):
            return np.ascontiguousarray(np.asarray(a, np.float32))

    def f32(a):
        return np.ascontiguousarray(np.asarray(a, np.float32))

    word = cast(word_hidden_states)
    ent = cast(entity_hidden_states)
    mask = f32(attention_mask).reshape(B, L)
    weights = {
        "qw": cast(qw), "kw": cast(kw), "vw": cast(vw),
        "w2e_qw": cast(w2e_qw), "w2e_kw": cast(w2e_kw),
        "e2w_qw": cast(e2w_qw), "e2w_kw": cast(e2w_kw),
    }
    biases = {
        "qb": f32(qb), "kb": f32(kb), "vb": f32(vb),
        "w2e_qb": f32(w2e_qb), "w2e_kb": f32(w2e_kb),
        "e2w_qb": f32(e2w_qb), "e2w_kb": f32(e2w_kb),
    }

    in_maps = []
    for c in range(N_CORES):
        sl = slice(BPC * c, BPC * (c + 1))
        m = {"word": word[sl], "ent": ent[sl], "mask": mask[sl]}
        m.update(weights)
        m.update(biases)
        in_maps.append(m)

    res = run_bass_kernel_spmd(nc, in_maps, core_ids=list(range(N_CORES)))
    word_out = np.concatenate([res.results[c]["out_w"] for c in range(N_CORES)], axis=0)
    ent_out = np.concatenate([res.results[c]["out_e"] for c in range(N_CORES)], axis=0)
    return word_out, ent_out
